# revision 1
# baseline (speedup 1.0000x reference)
"""Trainium2 Bass kernel for the dual-softmax cross-attention module.

Sharding: 8 cores = batch (4) x head-half (2).  Core c handles batch c//2 and
heads 4*(c%2) .. 4*(c%2)+4.  Each core computes Q/K/V projections for its
head-group, the 2048x2048 score matrix per head, one shared E = exp(s/8)
(both softmaxes are shift-invariant; scores are O(1) so no max subtraction),
contexts for both streams, exchanges context halves with its pair core via a
2-core AllGather, and produces a disjoint 256-channel slice of both outputs.

All matmuls run in bf16 (fp32 PSUM accumulation); residual + output stay fp32.
"""

import sys

for _p in ("/opt/trn_rl_repo", "/opt/pypackages"):
    if _p not in sys.path:
        sys.path.insert(0, _p)

import numpy as np
import ml_dtypes

import concourse.bass as bass
import concourse.tile as tile
from concourse import bacc, mybir
from concourse.bass_utils import run_bass_kernel_spmd

F32 = mybir.dt.float32
BF16 = mybir.dt.bfloat16
AF = mybir.ActivationFunctionType
AX = mybir.AxisListType

N_CORES = 8
B = 4          # batch
C = 512        # channels
N = 2048       # tokens (8*16*16)
H = 8          # heads
DH = 64        # head dim
HL = 4         # heads per core
CL = 256       # channels per core (head-group)
NT = N // 128  # 16 token tiles
CT = C // 128  # 4 channel tiles

_BF = ml_dtypes.bfloat16


def _build():
    nc = bacc.Bacc("TRN2", target_bir_lowering=False, debug=False,
                   num_devices=N_CORES)

    def din(name, shape, dt=BF16):
        return nc.dram_tensor(name, shape, dt, kind="ExternalInput").ap()

    x1b = din("x1b", [CT, 128, N])          # x1[b] channel-major, bf16
    x2b = din("x2b", [CT, 128, N])
    wq = din("wq", [128, CT, CL])           # column slice of Wq, pre-permuted
    wk = din("wk", [128, CT, CL])
    wv1 = din("wv1", [128, CT, CL])
    wv2 = din("wv2", [128, CT, CL])
    wo1 = din("wo1", [128, CT, CL])         # Wo columns for my output rows
    wo2 = din("wo2", [128, CT, CL])
    bq = din("bq", [128, 2, 1], F32)        # bias slices per M-tile
    bk = din("bk", [128, 2, 1], F32)
    bv1 = din("bv1", [1, CL])
    bv2 = din("bv2", [1, CL])
    x1r = din("x1r", [2, 128, N], F32)      # x1[b] residual slice + bo1
    x2r = din("x2r", [2, 128, N], F32)

    o1 = nc.dram_tensor("o1", [2, 128, N], F32, kind="ExternalOutput").ap()
    o2 = nc.dram_tensor("o2", [2, 128, N], F32, kind="ExternalOutput").ap()

    with tile.TileContext(nc) as tc:
        _emit(nc, tc, locals())
    nc.compile()
    return nc


def _emit(nc, tc, t):
    x1b, x2b = t["x1b"], t["x2b"]
    wq, wk, wv1, wv2 = t["wq"], t["wk"], t["wv1"], t["wv2"]
    wo1, wo2 = t["wo1"], t["wo2"]
    bq, bk, bv1, bv2 = t["bq"], t["bk"], t["bv1"], t["bv2"]
    x1r, x2r, o1, o2 = t["x1r"], t["x2r"], t["o1"], t["o2"]

    from contextlib import ExitStack
    ctx = ExitStack()
    with ctx:
        persist = ctx.enter_context(tc.tile_pool(name="persist", bufs=1))
        small = ctx.enter_context(tc.tile_pool(name="small", bufs=8))
        vp_pool = ctx.enter_context(tc.tile_pool(name="vp", bufs=4))
        dram = ctx.enter_context(tc.tile_pool(name="dram", bufs=2, space="DRAM"))

        # ---- persistent SBUF tensors (packed to dodge 4KB tile padding) ----
        w_all = persist.tile([128, 6, CT, CL], BF16, tag="wall")
        wq_s, wk_s, wv1_s, wv2_s, wo1_s, wo2_s = (w_all[:, i, :, :]
                                                  for i in range(6))
        bqk_s = persist.tile([128, 4, 1], F32, tag="bqk")
        bq_s, bk_s = bqk_s[:, 0:2, :], bqk_s[:, 2:4, :]
        ones_full = persist.tile([128, N], BF16, tag="ones", name="ones_full")
        ones_s = ones_full[0:1, :]
        misc_s = persist.tile([128, 640], BF16, tag="misc")
        bv1_s = misc_s[0:1, 0:CL]
        bv2_s = misc_s[0:1, CL:2 * CL]
        onec_s = misc_s[:, 512:513]
        qt_s = persist.tile([128, 2, N], BF16, tag="qt")    # Q^T  (chan-major)
        kt_s = persist.tile([128, 2, N], BF16, tag="kt")    # K^T
        v1tok = persist.tile([128, NT, CL], BF16, tag="v1tok")  # token-major V1
        v2tok = persist.tile([128, NT, CL], BF16, tag="v2tok")
        cm = {}  # gathered ctx^T tiles; pool opened once xb tiles retire

        for i, src in enumerate((wq, wk, wv1, wv2, wo1, wo2)):
            nc.sync.dma_start(w_all[:, i, :, :], src[:, :, :])
        nc.sync.dma_start(bq_s[:, :, :], bq[:, :, :])
        nc.sync.dma_start(bk_s[:, :, :], bk[:, :, :])
        nc.sync.dma_start(bv1_s[:, :], bv1[:, :])
        nc.sync.dma_start(bv2_s[:, :], bv2[:, :])
        nc.vector.memset(ones_s[:, :], 1.0)
        nc.vector.memset(onec_s[:, :], 1.0)

        # ---- P1: x loads + Q/K projections (V projections are interleaved
        # into head 0's qtile loop, using the then-idle ctx1 psum slot) ----
        # SBUF pool stacking: p2's SBUF pools open first, then xb (which is
        # released after head 0 so the gathered-context buffers reuse it).
        p2 = ExitStack()
        eslab = p2.enter_context(tc.tile_pool(name="eslab", bufs=6))
        et_pool = p2.enter_context(tc.tile_pool(name="et", bufs=1))
        gsrc_pool = p2.enter_context(tc.tile_pool(name="gsrc", bufs=2))
        csrow_pool = p2.enter_context(tc.tile_pool(name="csrow", bufs=1))
        p1 = ExitStack()
        pj_ps = p1.enter_context(tc.tile_pool(name="pj_ps", bufs=2, space="PSUM"))
        xb_stack = ExitStack()
        xb_pool = xb_stack.enter_context(tc.tile_pool(name="xb", bufs=8))
        xts = {}
        for xi, xb_dram in enumerate((x1b, x2b)):
            xts[xi] = [xb_pool.tile([128, N], BF16, tag="xb", name=f"xt{xi}_{i}")
                       for i in range(CT)]
            for ti in range(CT):
                nc.sync.dma_start(xts[xi][ti][:, :], xb_dram[ti, :, :])
        # chan-major Q/K:  out[cl, n] = sum_cin W[cin, cl] * x[cin, n]
        for xi, w_qk, b_qk, qk_dst in ((0, wq_s, bq_s, qt_s),
                                       (1, wk_s, bk_s, kt_s)):
            for m in range(2):
                for half in range(2):
                    ps = pj_ps.tile([128, 1024], F32, tag="pj")
                    for ch in range(2):
                        off = half * 1024 + ch * 512
                        for ti in range(CT):
                            nc.tensor.matmul(
                                ps[:, ch * 512:(ch + 1) * 512],
                                w_qk[:, ti, m * 128:(m + 1) * 128],
                                xts[xi][ti][:, off:off + 512],
                                start=(ti == 0), stop=(ti == CT - 1))
                    nc.scalar.activation(
                        qk_dst[:, m, half * 1024:(half + 1) * 1024], ps[:, :],
                        AF.Identity, bias=b_qk[:, m, :])
        p1.close()

        def emit_v_proj(xi, w_v, b_v, v_dst, nt, vps_pool):
            # token-major V:  out[n, cl] = sum_cin x[cin, n] * W[cin, cl] + bv
            ps = vps_pool.tile([128, 512], F32, tag="c1", name=f"vps{xi}_{nt}")
            for ti in range(CT):
                nc.tensor.matmul(
                    ps[:, 0:CL], xts[xi][ti][:, nt * 128:(nt + 1) * 128],
                    w_v[:, ti, :], start=(ti == 0), stop=False)
            nc.tensor.matmul(ps[:, 0:CL], ones_s[:, nt * 128:(nt + 1) * 128],
                             b_v[:, :], start=False, stop=True)
            nc.vector.tensor_copy(v_dst[:, nt, :], ps[:, 0:CL])

        # ---- P2: per-head attention, software-pipelined across heads ----
        # Per head hl, the qtile loop streams: scores -> exp(+rowsum chunk
        # accum) -> ctx2 (ones-augmented lhsT, so PSUM row 64 accumulates
        # colsum for free; emission lags one qtile) -> E^T transpose (lags 4).
        # Interleaved into head hl's loop is head hl-1's epilogue: ctx2 evac,
        # colsum row->column (16 K=1 matmuls), ctx1 spread ch-major over 8
        # qtiles (1-bank psum tiles; et stripes release per-ch), AllGather.
        sc_ps = p2.enter_context(tc.tile_pool(name="sc_ps", bufs=2, space="PSUM"))
        c2_ps = p2.enter_context(tc.tile_pool(name="c2_ps", bufs=1, space="PSUM"))
        c1_ps = p2.enter_context(tc.tile_pool(name="c1_ps", bufs=2, space="PSUM"))

        st = {}  # per-head pipeline state

        def head_slices(hl):
            g, poff = hl // 2, 64 * (hl % 2)
            return (qt_s[poff:poff + 64, g, :], kt_s[poff:poff + 64, g, :], poff)

        def emit_scores_exp(hl, qt):
            q_l, k_l, _ = head_slices(hl)
            s = st[hl]
            es = eslab.tile([128, N], BF16, tag="es", name=f"es{hl}_{qt}")
            sq = small.tile([128, 24], F32, tag="sq", bufs=4,
                            name=f"sq{hl}_{qt}")
            rs_p, rs, rr = sq[:, 0:3], sq[:, 4:5], sq[:, 5:6]
            for u in range(4):
                ps = sc_ps.tile([128, 512], F32, tag="sc", name=f"sps{u}")
                nc.tensor.matmul(ps[:, :], q_l[:, qt * 128:(qt + 1) * 128],
                                 k_l[:, u * 512:(u + 1) * 512],
                                 start=True, stop=True)
                # rowsum split: chunks 0-1 use the ACT fused accumulator,
                # chunks 2-3 are reduced on DVE in one op below
                nc.scalar.activation(es[:, u * 512:(u + 1) * 512], ps[:, :],
                                     AF.Exp, scale=0.125,
                                     accum_out=(rs_p[:, u:u + 1]
                                                if u < 2 else None))
            nc.vector.reduce_sum(out=rs_p[:, 2:3], in_=es[:, 1024:2048],
                                 axis=AX.X)
            nc.vector.reduce_sum(out=rs[:, :], in_=rs_p[:, :], axis=AX.X)
            nc.vector.reciprocal(rr[:, :], rs[:, :])
            if qt % 4 == 0:
                s["v2pk"] = vp_pool.tile([128, 4, DH + 1], BF16, tag="v2p",
                                         bufs=2, name=f"v2pk{hl}_{qt}")
            v2p = s["v2pk"][:, qt % 4, :]
            nc.vector.tensor_scalar_mul(
                v2p[:, 0:DH], v2tok[:, qt, hl * DH:(hl + 1) * DH], rr[:, :])
            nc.vector.memset(v2p[:, DH:DH + 1], 1.0)
            s["es"][qt] = es
            s["v2p"][qt] = v2p

        def emit_ctx2(hl, qt):
            s = st[hl]
            for ch in range(4):
                nc.tensor.matmul(
                    s["cps2"][0:DH + 1, ch * 512:(ch + 1) * 512],
                    s["v2p"][qt][:, :], s["es"][qt][:, ch * 512:(ch + 1) * 512],
                    start=(qt == 0), stop=(qt == NT - 1))

        def emit_transpose(hl, qt):
            s = st[hl]
            if s["et"] is None:
                s["et"] = et_pool.tile([128, NT, N], BF16, tag="et",
                                       name=f"et{hl}")
            nc.sync.dma_start(
                s["et"][:, qt, :].rearrange("p (a b) -> p a b", b=128),
                s["es"][qt][:, :], transpose=True)

        def emit_epilogue_a(hl):
            # copy colsum row out of psum FIRST (it gates the next head's
            # colsum matmuls / ctx1 chain on the PE), then evac ctx2
            s = st[hl]
            csrow = csrow_pool.tile([65, N], BF16, tag="csr", name=f"csr{hl}")
            s["csrow"] = csrow
            nc.vector.tensor_copy(csrow[64:65, :], s["cps2"][64:65, :])
            gs = gsrc_pool.tile([128, N], BF16, tag="gs", name=f"gs{hl}")
            s["gs"] = gs
            nc.vector.tensor_copy(gs[0:64, :], s["cps2"][0:64, :])

        def emit_epilogue_b(hl):
            # colsum row -> column via 16 K=1 matmuls, recip, scale v1
            s = st[hl]
            cs_ps = sc_ps.tile([128, 512], F32, tag="sc", name=f"cs_ps{hl}")
            for kt in range(NT):
                nc.tensor.matmul(cs_ps[:, kt:kt + 1],
                                 s["csrow"][64:65, kt * 128:(kt + 1) * 128],
                                 onec_s[64:65, :], start=True, stop=True)
            cr_t = small.tile([128, NT], F32, tag="cr", bufs=2, name=f"cr{hl}")
            nc.vector.reciprocal(cr_t[:, :], cs_ps[:, 0:NT])
            v1pk = vp_pool.tile([128, NT, DH], BF16, tag="v1p", bufs=2,
                                name=f"v1pk{hl}")
            for kt in range(NT):
                nc.vector.tensor_scalar_mul(
                    v1pk[:, kt, :], v1tok[:, kt, hl * DH:(hl + 1) * DH],
                    cr_t[:, kt:kt + 1])
                s["v1p"][kt] = v1pk[:, kt, :]

        def emit_ctx1_step(hl, step):
            # step 0..11: ch = step//3, kt third = step%3 (6/5/5 kts)
            s = st[hl]
            ch, third = step // 3, step % 3
            kt_lo, kt_hi = (0, 6) if third == 0 else (
                (6, 11) if third == 1 else (11, 16))
            if third == 0:
                s["c1"][ch] = c1_ps.tile([128, 512], F32, tag="c1",
                                         name=f"c1_{hl}_{ch}")
            for kt in range(kt_lo, kt_hi):
                nc.tensor.matmul(
                    s["c1"][ch][64:128, :], s["v1p"][kt][:, :],
                    s["et"][:, 4 * ch:4 * (ch + 1), kt * 128:(kt + 1) * 128],
                    start=(kt == 0), stop=(kt == NT - 1))
            if third == 2:
                nc.vector.tensor_copy(
                    s["gs"][64:128, ch * 512:(ch + 1) * 512],
                    s["c1"][ch][64:128, :])

        def emit_gather(hl, half=None):
            # half=None: gather both ctx halves; 0: ctx2 rows only; 1: ctx1
            s = st[hl]
            _, _, poff = head_slices(hl)
            rows = slice(0, 128) if half is None else (
                slice(0, 64) if half == 0 else slice(64, 128))
            nr = rows.stop - rows.start
            sfx = f"{hl}_{half}"
            gin = dram.tile([nr, N], BF16, tag="gin", name=f"gin{sfx}")
            gout = dram.tile([2, nr, N], BF16, tag="gout", bufs=4,
                             name=f"gout{sfx}")
            nc.gpsimd.dma_start(gin[:, :], s["gs"][rows, :])
            nc.gpsimd.collective_compute(
                "AllGather", mybir.AluOpType.bypass,
                replica_groups=[[0, 1], [2, 3], [4, 5], [6, 7]],
                ins=[gin.opt()], outs=[gout.opt()])
            for r in range(2):
                tt = 2 * r + hl // 2
                if half in (None, 0):
                    nc.sync.dma_start(cm["2"][poff:poff + 64, tt, :],
                                      gout[r, 0:64, :])
                if half in (None, 1):
                    ro = 64 if half is None else 0
                    nc.sync.dma_start(cm["1"][poff:poff + 64, tt, :],
                                      gout[r, ro:ro + 64, :])

        def emit_head_qt(hl, qt):
            # one qtile of head hl + interleaved epilogue work of head hl-1
            # (or, for head 0, the V projections)
            if hl == 0:
                emit_v_proj(1, wv2_s, bv2_s, v2tok, qt, c1_ps)
            emit_scores_exp(hl, qt)
            if hl == 0:
                emit_v_proj(0, wv1_s, bv1_s, v1tok, qt, c1_ps)
            else:
                if qt == 1:
                    emit_epilogue_b(hl - 1)
                elif 2 <= qt <= 13:
                    emit_ctx1_step(hl - 1, qt - 2)
                elif qt == 14:
                    emit_gather(hl - 1)
            if qt > 0:
                emit_ctx2(hl, qt - 1)
            if qt >= 4:
                emit_transpose(hl, qt - 4)

        for hl in range(HL):
            st[hl] = {"es": {}, "v2p": {}, "v1p": {}, "c1": {}, "et": None,
                      "cps2": c2_ps.tile([128, N], F32, tag="c2",
                                         name=f"cps2_{hl}")}
            for qt in range(NT):
                emit_head_qt(hl, qt)
            emit_ctx2(hl, NT - 1)
            emit_epilogue_a(hl)
            for qt in range(NT - 4, NT):
                emit_transpose(hl, qt)
            if hl == 0:
                # x tiles retire with head 0's V projections; reuse their
                # SBUF for the gathered-context buffers
                xb_stack.close()
                cm_pool = p2.enter_context(tc.tile_pool(name="cm", bufs=1))
                cm["1"] = cm_pool.tile([128, CT, N], BF16, tag="ctxm1",
                                       name="ctxm1")
                cm["2"] = cm_pool.tile([128, CT, N], BF16, tag="ctxm2",
                                       name="ctxm2")
        # epilogue of the last head: ship the ctx2 half while ctx1 computes
        emit_gather(HL - 1, half=0)
        emit_epilogue_b(HL - 1)
        for step in range(12):
            emit_ctx1_step(HL - 1, step)
        emit_gather(HL - 1, half=1)

        p2.close()

        # ---- P3: output projections + residual ----
        p3 = ExitStack()
        o_ps = p3.enter_context(tc.tile_pool(name="o_ps", bufs=2, space="PSUM"))
        xr_pool = p3.enter_context(tc.tile_pool(name="xr", bufs=2))
        out_pool = p3.enter_context(tc.tile_pool(name="outp", bufs=2))
        for w_s, cmt, xr, oo in ((wo2_s, cm["2"], x2r, o2),
                                 (wo1_s, cm["1"], x1r, o1)):
            for m in range(2):
                xr_t = xr_pool.tile([128, N], F32, tag="xr")
                nc.sync.dma_start(xr_t[:, :], xr[m, :, :])
                ps = o_ps.tile([128, N], F32, tag="o")
                # tiles 0,2 hold heads 0-5 (ready after gather(1)); tiles
                # 1,3 need the last gather -- accumulate those last
                for tis in ((0, 2), (1, 3)):
                    for ch in range(4):
                        for ti in tis:
                            nc.tensor.matmul(
                                ps[:, ch * 512:(ch + 1) * 512],
                                w_s[:, ti, m * 128:(m + 1) * 128],
                                cmt[:, ti, ch * 512:(ch + 1) * 512],
                                start=(ti == 0), stop=(ti == 3))
                ot = out_pool.tile([128, N], F32, tag="ot")
                nc.vector.tensor_add(ot[:, :], ps[:, :], xr_t[:, :])
                nc.sync.dma_start(oo[m, :, :], ot[:, :])
        p3.close()


_NC_CACHE = None


def _get_nc():
    global _NC_CACHE
    if _NC_CACHE is None:
        _NC_CACHE = _build()
    return _NC_CACHE


def _in_maps(x1, x2, Wq, bq, Wk, bk, Wv1, bv1, Wv2, bv2, Wo1, bo1, Wo2, bo2):
    x1f = np.asarray(x1, np.float32).reshape(B, C, N)
    x2f = np.asarray(x2, np.float32).reshape(B, C, N)
    in_maps = []
    for c in range(N_CORES):
        b, hq = c // 2, c % 2
        sl = slice(CL * hq, CL * hq + CL)
        def wslice(W):
            return np.ascontiguousarray(
                np.asarray(W, np.float32)[:, sl].reshape(CT, 128, CL)
                .transpose(1, 0, 2)).astype(_BF)

        m = {
            "x1b": x1f[b].reshape(CT, 128, N).astype(_BF),
            "x2b": x2f[b].reshape(CT, 128, N).astype(_BF),
            "wq": wslice(Wq), "wk": wslice(Wk),
            "wv1": wslice(Wv1), "wv2": wslice(Wv2),
            "wo1": wslice(Wo1), "wo2": wslice(Wo2),
            "bq": np.ascontiguousarray(
                np.asarray(bq, np.float32)[sl].reshape(2, 128).T).reshape(128, 2, 1),
            "bk": np.ascontiguousarray(
                np.asarray(bk, np.float32)[sl].reshape(2, 128).T).reshape(128, 2, 1),
            "bv1": np.asarray(bv1, np.float32)[sl].reshape(1, CL).astype(_BF),
            "bv2": np.asarray(bv2, np.float32)[sl].reshape(1, CL).astype(_BF),
            "x1r": (x1f[b, sl, :] + np.asarray(bo1, np.float32)[sl, None]
                    ).reshape(2, 128, N),
            "x2r": (x2f[b, sl, :] + np.asarray(bo2, np.float32)[sl, None]
                    ).reshape(2, 128, N),
        }
        in_maps.append(m)
    return in_maps


def _unshard(res):
    o1 = np.empty((B, C, N), np.float32)
    o2 = np.empty((B, C, N), np.float32)
    for c in range(N_CORES):
        b, hq = c // 2, c % 2
        sl = slice(CL * hq, CL * hq + CL)
        o1[b, sl, :] = res[c]["o1"].reshape(CL, N)
        o2[b, sl, :] = res[c]["o2"].reshape(CL, N)
    shape = (B, C, 8, 16, 16)
    return o1.reshape(shape), o2.reshape(shape)


def kernel(**inputs):
    in_maps = _in_maps(**inputs)
    nc = _get_nc()
    res = run_bass_kernel_spmd(nc, in_maps, list(range(N_CORES))).results
    return _unshard(res)

